# revision 14
# baseline (speedup 1.0000x reference)
"""Multi-head self-attention (RoPE, causal) TRN2 Bass kernel.

Problem: B=4, S=2048, D=1024, H=16, Dh=64, fp32.

Sharding (8 cores): DP=4 over batch x TP=2 over heads (Megatron-style).
Core c handles batch c//2 with heads (c%2)*8 .. (c%2)*8+7 and produces a
partial output [S, D] (stored transposed); the host sums the two TP
partials per batch (the all-reduce after out_projection).

Per-core device program (all matmuls in float32r = TF32-like single-pass):
  Phase 1: QKV projection from host-transposed X^T/W^T. Q^T/K^T in [e, t]
    layout (RoPE pair components pre-permuted to [x0|x1] via W row perm),
    V natural [t, dv] with an appended ones-column (denominator trick).
    RoPE in-place: partition-swap via SBUF DMA + 3 DVE ops.
  Phase 2: attention with transposed scores, head pairs row-packed on the
    PE (rows 0-63 / 64-127). S^T = K_blk^T.T @ Q^T -> one ACT exp (both
    heads, no max subtraction; scores bounded) -> triangular mask on
    diagonal blocks -> ctx_aug[65, q] += V_aug.T @ P^T. Row 64 = softmax
    denominator. The (i, p) loops are interleaved pair-wise; the tail of
    each unit is a single [65,512] PSUM->SBUF stash into dead Q^T columns
    (even heads) or a ctx tile (odd heads).
  Phase 2.5: all 32 denominator rows DMA-gathered into one [32,512] tile,
    one batched reciprocal_approx_fast, one-hot selector matmuls broadcast
    each recip row, DVE multiply normalizes ctx in place.
  Phase 3: out projection, W_out chunk stationary: outT[e, t] += wo.T@ctx
    accumulated over the 8 local heads (K=64), host transposes back.
"""

import sys

for _p in ("/opt/trn_rl_repo", "/root/.axon_site/_ro/trn_rl_repo"):
    if _p not in sys.path:
        sys.path.insert(0, _p)

import numpy as np

import concourse.bacc as bacc
import concourse.bass_utils as bass_utils
import concourse.mybir as mybir
import concourse.tile as tile
from concourse.bass_utils import run_bass_kernel_spmd

# Allow walrus to elide redundant LDWEIGHTS (stationary-operand reuse);
# K=64 matmuls cannot hide same-row-group weight reloads otherwise.
if not getattr(bass_utils, "_ldw_opt_patched", False):
    _orig_run_command = bass_utils.run_command

    def _run_command_ldw(argv, **kwargs):
        argv = [
            "--enable-ldw-opt=true" if a == "--enable-ldw-opt=false" else a
            for a in argv
        ]
        return _orig_run_command(argv, **kwargs)

    bass_utils.run_command = _run_command_ldw
    bass_utils._ldw_opt_patched = True

F32 = mybir.dt.float32
F32R = mybir.dt.float32r
EXP = mybir.ActivationFunctionType.Exp

B, S, D = 4, 2048, 1024
H, DH = 16, 64
THETA = 10000.0
NCORES, TP, HLOC = 8, 2, 8          # 8 local heads per core, 4 pairs
NPAIR = HLOC // 2
NT = S // 512                        # 4 q/t tiles of 512
NTQ = S // 128                       # 16 t-chunks of 128
ND = D // 128                        # 8 d-chunks
SCALE = 1.0 / 8.0                    # 1/sqrt(DH)

_PROGRAM = None


def _build_program():
    nc = bacc.Bacc(None)

    xT_d = nc.dram_tensor("xT", [D, S], F32R, kind="ExternalInput")
    wqkvT_d = nc.dram_tensor("wqkvT", [D, 3 * HLOC * DH], F32R, kind="ExternalInput")
    woT_d = nc.dram_tensor("woT", [NPAIR, 128, D], F32R, kind="ExternalInput")
    cos_d = nc.dram_tensor("cosT", [128, S], F32, kind="ExternalInput")
    sin_d = nc.dram_tensor("sinT", [128, S], F32, kind="ExternalInput")
    mask_d = nc.dram_tensor("mask", [128, 128], F32, kind="ExternalInput")
    onehot_d = nc.dram_tensor("onehot", [8, 8 * 64], F32R, kind="ExternalInput")
    out_d = nc.dram_tensor("out", [D, S], F32, kind="ExternalOutput")

    with tile.TileContext(nc) as tc:
        with (
            tc.tile_pool(name="const", bufs=1) as constp,
            tc.tile_pool(name="vpool", bufs=1) as vpool,
            tc.tile_pool(name="qkpool", bufs=1) as qkpool,
        ):
            mask_sb = constp.tile([128, 128], F32)
            nc.sync.dma_start(mask_sb[:], mask_d[:])
            ones8 = constp.tile([128, 8], F32)
            nc.vector.memset(ones8[:], 1.0)

            qt = [qkpool.tile([128, S], F32R, name=f"qt{p}") for p in range(NPAIR)]
            kt = [qkpool.tile([128, S], F32R, name=f"kt{p}") for p in range(NPAIR)]
            vt = [vpool.tile([128, HLOC, DH + 1], F32R, name=f"v{t}") for t in range(NTQ)]

            # ---------------- Phase 1: QKV projection + RoPE ----------------
            with (
                tc.tile_pool(name="wpool", bufs=1) as wpool,
                tc.tile_pool(name="xpool", bufs=1) as xpool,
                tc.tile_pool(name="cspool", bufs=1) as cspool,
                tc.tile_pool(name="ropep", bufs=1) as ropep,
                tc.tile_pool(name="ps1", bufs=1, space="PSUM") as ps1,
            ):
                xTr = xT_d.rearrange("(d p) t -> p d t", p=128)
                w_sb = [wpool.tile([128, 3 * HLOC * DH], F32R, name=f"w{d}")
                        for d in range(ND)]

                for ts in range(NT):
                    tsl = slice(ts * 512, (ts + 1) * 512)
                    xa = xpool.tile([128, ND, 512], F32R, tag="x", bufs=2)
                    nc.sync.dma_start(xa[:], xTr[:, :, tsl])
                    if ts == 0:
                        for d in range(ND):
                            nc.sync.dma_start(
                                w_sb[d][:], wqkvT_d[d * 128:(d + 1) * 128, :])
                    cos_sb = cspool.tile([128, 512], F32, tag="cos", bufs=2)
                    sin_sb = cspool.tile([128, 512], F32, tag="sin", bufs=2)
                    nc.sync.dma_start(cos_sb[:], cos_d[:, tsl])
                    nc.sync.dma_start(sin_sb[:], sin_d[:, tsl])

                    # Q^T and K^T e-chunks (e = pair for Q, 4+pair for K)
                    for e in range(2 * NPAIR):
                        ps = ps1.tile([128, 512], F32, tag="qkps", bufs=3)
                        for d in range(ND):
                            nc.tensor.matmul(
                                ps[:], w_sb[d][:, e * 128:(e + 1) * 128],
                                xa[:, d, :],
                                start=(d == 0), stop=(d == ND - 1),
                            )
                        dst = qt[e] if e < NPAIR else kt[e - NPAIR]
                        nc.any.tensor_copy(dst[:, tsl], ps[:])
                        # RoPE in place: quadrant swap via SBUF->SBUF DMA
                        sw = ropep.tile([128, 512], F32, tag="sw", bufs=2)
                        for qd in range(4):
                            sq = qd ^ 1
                            nc.gpsimd.dma_start(
                                sw[qd * 32:(qd + 1) * 32, :],
                                dst.bitcast(F32)[sq * 32:(sq + 1) * 32, tsl],
                            )
                        t1 = ropep.tile([128, 512], F32, tag="t1", bufs=2)
                        nc.vector.tensor_mul(t1[:], dst[:, tsl], cos_sb[:])
                        nc.vector.tensor_mul(sw[:], sw[:], sin_sb[:])
                        nc.vector.tensor_add(dst[:, tsl], t1[:], sw[:])

                    # V (natural layout), 4 t-chunks of 128 per ts
                    for tq0 in range(4):
                        tq = ts * 4 + tq0
                        psv = ps1.tile([128, 512], F32, tag="qkps", bufs=3)
                        for d in range(ND):
                            nc.tensor.matmul(
                                psv[:],
                                xa[:, d, tq0 * 128:(tq0 + 1) * 128],
                                w_sb[d][:, 2 * HLOC * DH:3 * HLOC * DH],
                                start=(d == 0), stop=(d == ND - 1),
                            )
                        v = vt[tq]
                        nc.any.tensor_copy(
                            v[:, :, 0:DH],
                            psv.rearrange("p (h d) -> p h d", h=HLOC),
                        )
                        nc.any.tensor_copy(v[:, :, DH:DH + 1], ones8[:, :, None])

            with tc.tile_pool(name="ctxbp", bufs=1) as ctxbp:
                # ctx homes for odd heads; even heads reuse dead qt columns
                ctxb = [ctxbp.tile([65, S], F32R, name=f"ctxb{p}")
                        for p in range(NPAIR)]

                def ctx_home(h):
                    p = h // 2
                    return qt[p] if h % 2 == 0 else ctxb[p]

                # ---------------- Phase 2: attention ----------------
                with (
                    tc.tile_pool(name="ptpool", bufs=1) as ptpool,
                    tc.tile_pool(name="nrmpool", bufs=1) as nrmpool,
                    tc.tile_pool(name="stp", bufs=1, space="PSUM") as stp,
                    tc.tile_pool(name="cdp", bufs=1, space="PSUM") as cdp,
                ):
                    onehot_sb = nrmpool.tile([8, 8 * 64], F32R)
                    nc.sync.dma_start(onehot_sb[:], onehot_d[:])
                    for i in range(NT):
                        for p in range(NPAIR):
                            ctxA = cdp.tile([65, 512], F32, tag="ctx", bufs=4)
                            ctxB = cdp.tile([65, 512], F32, tag="ctx", bufs=4)
                            nj = 4 * i + 4
                            for j in range(nj):
                                lo = max(0, 128 * j - 512 * i)
                                qsl = slice(512 * i + lo, 512 * (i + 1))
                                ksl = slice(j * 128, (j + 1) * 128)
                                st = stp.tile([128, 2, 512], F32, tag="st", bufs=2)
                                nc.tensor.matmul(
                                    st[:, 0, lo:512], kt[p][0:64, ksl],
                                    qt[p][0:64, qsl], tile_position=(0, 0),
                                )
                                nc.tensor.matmul(
                                    st[:, 1, lo:512], kt[p][64:128, ksl],
                                    qt[p][64:128, qsl], tile_position=(64, 0),
                                )
                                pt = ptpool.tile([128, 2, 512], F32R, tag="pt", bufs=6)
                                nc.scalar.activation(
                                    pt[:, :, lo:512], st[:, :, lo:512], EXP,
                                    scale=SCALE,
                                )
                                if lo == 128 * j - 512 * i:  # starts on diagonal
                                    nc.vector.tensor_mul(
                                        pt[:, :, lo:lo + 128],
                                        pt[:, :, lo:lo + 128],
                                        mask_sb[:, None, :].to_broadcast([128, 2, 128]),
                                    )
                                nc.tensor.matmul(
                                    ctxA[:, lo:512], vt[j][:, 2 * p, :],
                                    pt[:, 0, lo:512],
                                    start=(j == 0), stop=(j == nj - 1),
                                )
                                nc.tensor.matmul(
                                    ctxB[:, lo:512], vt[j][:, 2 * p + 1, :],
                                    pt[:, 1, lo:512],
                                    start=(j == 0), stop=(j == nj - 1),
                                )
                            isl = slice(512 * i, 512 * (i + 1))
                            # stash unnormalized ctx + denominator row
                            nc.vector.tensor_copy(qt[p][0:65, isl], ctxA[:])
                            nc.vector.tensor_copy(ctxb[p][:, isl], ctxB[:])

                        # per-q-tile softmax normalization for row i, emitted
                        # inline so it overlaps the next attention row
                        tsl = isl
                        den_g = nrmpool.tile([8, 512], F32R, tag="deng", bufs=2,
                                             name=f"deng{i}")
                        for h in range(HLOC):
                            nc.sync.dma_start(
                                den_g[h:h + 1, :], ctx_home(h)[64:65, tsl])
                        rec = nrmpool.tile([8, 512], F32, tag="recg", bufs=2,
                                           name=f"rec{i}")
                        nc.vector.reciprocal_approx_fast(
                            rec[:], den_g.bitcast(F32)[:])
                        rec_r = nrmpool.tile([8, 512], F32R, tag="recr", bufs=2,
                                             name=f"recr{i}")
                        nc.vector.tensor_copy(rec_r[:], rec[:])
                        for h in range(HLOC):
                            home = ctx_home(h)
                            bc = cdp.tile([64, 512], F32, tag="ctx", bufs=4,
                                          name=f"bc{i}_{h}")
                            nc.tensor.matmul(
                                bc[:], onehot_sb[:, h * 64:(h + 1) * 64], rec_r[:])
                            bc_sb = nrmpool.tile([64, 512], F32, tag="bcsb", bufs=4)
                            nc.any.tensor_copy(bc_sb[:], bc[:])
                            nc.vector.tensor_mul(
                                home[0:64, tsl], home[0:64, tsl], bc_sb[:])
                        # repack odd-head ctx into dead Q_B rows (K=128 out-proj)
                        for p in range(NPAIR):
                            nc.sync.dma_start(
                                qt[p][64:128, tsl], ctxb[p][0:64, tsl])

                # ---------- Phase 3: out projection (outT layout) ----------
                with (
                    tc.tile_pool(name="wopool", bufs=1) as wopool,
                    tc.tile_pool(name="otpool", bufs=1) as otpool,
                    tc.tile_pool(name="pso", bufs=1, space="PSUM") as psop,
                ):
                    wo_sb = []
                    for p in range(NPAIR):
                        wo = wopool.tile([128, D], F32R, name=f"wo{p}")
                        nc.sync.dma_start(wo[:], woT_d[p])
                        wo_sb.append(wo)

                    for ts in range(NT):
                        tsl = slice(ts * 512, (ts + 1) * 512)
                        for ec in range(D // 128):
                            ecs = slice(ec * 128, (ec + 1) * 128)
                            pso = psop.tile([128, 512], F32, tag="o", bufs=6,
                                            name=f"pso_{ts}_{ec}")
                            for p in range(NPAIR):
                                nc.tensor.matmul(
                                    pso[:], wo_sb[p][:, ecs], qt[p][:, tsl],
                                    start=(p == 0), stop=(p == NPAIR - 1),
                                )
                            ot = otpool.tile([128, 512], F32, tag="ot", bufs=4)
                            nc.vector.tensor_copy(ot[:], pso[:])
                            nc.sync.dma_start(out_d[ecs, tsl], ot[:])

    nc.compile()
    return nc


def _get_program():
    global _PROGRAM
    if _PROGRAM is None:
        _PROGRAM = _build_program()
    return _PROGRAM


def _prep_in_maps(in_features, token_positions, W_qkv, W_out):
    in_features = np.asarray(in_features, dtype=np.float32)
    token_positions = np.asarray(token_positions)
    W_qkv = np.asarray(W_qkv, dtype=np.float32)
    W_out = np.asarray(W_out, dtype=np.float32)

    # RoPE pair permutation: [x0 of freq 0..31 | x1 of freq 0..31]
    perm = np.concatenate([np.arange(0, DH, 2), np.arange(1, DH, 2)])

    wqkvT, woT = [], []
    for tp in range(TP):
        rows = []
        for sect in range(2):  # Q, K (permuted)
            for h in range(HLOC):
                g = tp * HLOC + h
                rows.append(W_qkv[sect * D + g * DH + perm])
        for h in range(HLOC):  # V natural
            g = tp * HLOC + h
            rows.append(W_qkv[2 * D + g * DH:2 * D + (g + 1) * DH])
        Wl = np.concatenate(rows, axis=0)  # [1536, 1024]
        wqkvT.append(np.ascontiguousarray(Wl.T))
        woT.append(np.ascontiguousarray(np.stack(
            [np.concatenate([
                W_out[:, (tp * HLOC + 2 * p) * DH:(tp * HLOC + 2 * p + 1) * DH].T,
                W_out[:, (tp * HLOC + 2 * p + 1) * DH:(tp * HLOC + 2 * p + 2) * DH].T,
            ], axis=0) for p in range(NPAIR)])))

    half = DH // 2
    inv_freq = (THETA ** (-2.0 * np.arange(half, dtype=np.float32) / DH)).astype(np.float32)
    ang = token_positions.astype(np.float32)[:, None] * inv_freq[None, :]  # [S, 32]
    cos_t = np.cos(ang).T.astype(np.float32)  # [32, S]
    sin_t = np.sin(ang).T.astype(np.float32)
    cos128 = np.ascontiguousarray(np.tile(cos_t, (4, 1)))
    sin128 = np.ascontiguousarray(np.tile(np.concatenate([-sin_t, sin_t], axis=0), (2, 1)))
    # mask[kv, c] = 1 iff kv <= c (scores stored transposed: [kv, q])
    mask128 = np.triu(np.ones((128, 128), dtype=np.float32))
    onehot = np.zeros((8, 8 * 64), dtype=np.float32)
    for k in range(8):
        onehot[k, k * 64:(k + 1) * 64] = 1.0

    in_maps = []
    for c in range(NCORES):
        b, tp = c // 2, c % 2
        in_maps.append({
            "xT": np.ascontiguousarray(in_features[b].T),
            "wqkvT": wqkvT[tp],
            "woT": woT[tp],
            "cosT": cos128,
            "sinT": sin128,
            "mask": mask128,
            "onehot": onehot,
        })
    return in_maps


def run(in_features, token_positions, W_qkv, W_out, **spmd_kwargs):
    """Run the kernel; returns (output [B,S,D] f32, BassKernelResults)."""
    in_maps = _prep_in_maps(in_features, token_positions, W_qkv, W_out)
    nc = _get_program()
    res = run_bass_kernel_spmd(nc, in_maps, core_ids=list(range(NCORES)), **spmd_kwargs)
    outs = [res.results[c]["out"] for c in range(NCORES)]
    full = np.stack([(outs[2 * b] + outs[2 * b + 1]).T for b in range(B)])
    return full.astype(np.float32), res


def kernel(in_features, token_positions, W_qkv, W_out):
    out, _ = run(in_features, token_positions, W_qkv, W_out)
    return out


# revision 15
# speedup vs baseline: 1.1640x; 1.1640x over previous
"""Multi-head self-attention (RoPE, causal) TRN2 Bass kernel.

Problem: B=4, S=2048, D=1024, H=16, Dh=64, fp32.

Sharding (8 cores): DP=4 over batch x TP=2 over heads (Megatron-style).
Core c handles batch c//2 with heads (c%2)*8 .. (c%2)*8+7 and produces a
partial output [S, D] (stored transposed); the host sums the two TP
partials per batch (the all-reduce after out_projection).

Per-core device program (all matmuls in float32r = TF32-like single-pass):
  Phase 1: QKV projection from host-transposed X^T/W^T. Q^T/K^T in [e, t]
    layout (RoPE pair components pre-permuted to [x0|x1] via W row perm),
    V natural [t, dv] with an appended ones-column (denominator trick).
    RoPE in-place: partition-swap via SBUF DMA + 3 DVE ops.
  Phase 2: attention with transposed scores, head pairs row-packed on the
    PE (rows 0-63 / 64-127). S^T = K_blk^T.T @ Q^T -> one ACT exp (both
    heads, no max subtraction; scores bounded) -> triangular mask on
    diagonal blocks -> ctx_aug[65, q] += V_aug.T @ P^T. Row 64 = softmax
    denominator. The (i, p) loops are interleaved pair-wise; the tail of
    each unit is a single [65,512] PSUM->SBUF stash into dead Q^T columns
    (even heads) or a ctx tile (odd heads).
  Phase 2.5: all 32 denominator rows DMA-gathered into one [32,512] tile,
    one batched reciprocal_approx_fast, one-hot selector matmuls broadcast
    each recip row, DVE multiply normalizes ctx in place.
  Phase 3: out projection, W_out chunk stationary: outT[e, t] += wo.T@ctx
    accumulated over the 8 local heads (K=64), host transposes back.
"""

import sys

for _p in ("/opt/trn_rl_repo", "/root/.axon_site/_ro/trn_rl_repo"):
    if _p not in sys.path:
        sys.path.insert(0, _p)

import numpy as np

import concourse.bacc as bacc
import concourse.bass_utils as bass_utils
import concourse.mybir as mybir
import concourse.tile as tile
from concourse.bass_utils import run_bass_kernel_spmd

# Allow walrus to elide redundant LDWEIGHTS (stationary-operand reuse);
# K=64 matmuls cannot hide same-row-group weight reloads otherwise.
if not getattr(bass_utils, "_ldw_opt_patched", False):
    _orig_run_command = bass_utils.run_command

    def _run_command_ldw(argv, **kwargs):
        argv = [
            "--enable-ldw-opt=true" if a == "--enable-ldw-opt=false" else a
            for a in argv
        ]
        return _orig_run_command(argv, **kwargs)

    bass_utils.run_command = _run_command_ldw
    bass_utils._ldw_opt_patched = True

F32 = mybir.dt.float32
F32R = mybir.dt.float32r
EXP = mybir.ActivationFunctionType.Exp

B, S, D = 4, 2048, 1024
H, DH = 16, 64
THETA = 10000.0
NCORES, TP, HLOC = 8, 2, 8          # 8 local heads per core, 4 pairs
NPAIR = HLOC // 2
NT = S // 512                        # 4 q/t tiles of 512
NTQ = S // 128                       # 16 t-chunks of 128
ND = D // 128                        # 8 d-chunks
SCALE = 1.0 / 8.0                    # 1/sqrt(DH)

_PROGRAM = None


def _build_program():
    nc = bacc.Bacc(None)

    xT_d = nc.dram_tensor("xT", [D, S], F32R, kind="ExternalInput")
    wqkvT_d = nc.dram_tensor("wqkvT", [D, 3 * HLOC * DH], F32R, kind="ExternalInput")
    woT_d = nc.dram_tensor("woT", [NPAIR, 128, D], F32R, kind="ExternalInput")
    cos_d = nc.dram_tensor("cosT", [128, S], F32, kind="ExternalInput")
    sin_d = nc.dram_tensor("sinT", [128, S], F32, kind="ExternalInput")
    mask_d = nc.dram_tensor("mask", [128, 128], F32, kind="ExternalInput")
    onehot_d = nc.dram_tensor("onehot", [8, 8 * 64], F32R, kind="ExternalInput")
    out_d = nc.dram_tensor("out", [D, S], F32, kind="ExternalOutput")

    with tile.TileContext(nc) as tc:
        with (
            tc.tile_pool(name="const", bufs=1) as constp,
            tc.tile_pool(name="vpool", bufs=1) as vpool,
            tc.tile_pool(name="qkpool", bufs=1) as qkpool,
        ):
            mask_sb = constp.tile([128, 128], F32)
            nc.sync.dma_start(mask_sb[:], mask_d[:])
            ones8 = constp.tile([128, 8], F32)
            nc.vector.memset(ones8[:], 1.0)

            qt = [qkpool.tile([128, S], F32R, name=f"qt{p}") for p in range(NPAIR)]
            kt = [qkpool.tile([128, S], F32R, name=f"kt{p}") for p in range(NPAIR)]
            vt = [vpool.tile([128, HLOC, DH + 1], F32R, name=f"v{t}") for t in range(NTQ)]

            # ---------------- Phase 1: QKV projection + RoPE ----------------
            with (
                tc.tile_pool(name="wpool", bufs=1) as wpool,
                tc.tile_pool(name="xpool", bufs=1) as xpool,
                tc.tile_pool(name="cspool", bufs=1) as cspool,
                tc.tile_pool(name="ropep", bufs=1) as ropep,
                tc.tile_pool(name="ps1", bufs=1, space="PSUM") as ps1,
            ):
                xTr = xT_d.rearrange("(d p) t -> p d t", p=128)
                w_sb = [wpool.tile([128, 3 * HLOC * DH], F32R, name=f"w{d}")
                        for d in range(ND)]

                for ts in range(NT):
                    tsl = slice(ts * 512, (ts + 1) * 512)
                    xa = xpool.tile([128, ND, 512], F32R, tag="x", bufs=2)
                    for d in range(ND):
                        nc.sync.dma_start(xa[:, d, :],
                                          xT_d[d * 128:(d + 1) * 128, tsl])
                    if ts == 0:
                        for d in range(ND):
                            nc.sync.dma_start(
                                w_sb[d][:], wqkvT_d[d * 128:(d + 1) * 128, :])
                    cos_sb = cspool.tile([128, 512], F32, tag="cos", bufs=2)
                    sin_sb = cspool.tile([128, 512], F32, tag="sin", bufs=2)
                    nc.sync.dma_start(cos_sb[:], cos_d[:, tsl])
                    nc.sync.dma_start(sin_sb[:], sin_d[:, tsl])

                    # Q^T and K^T e-chunks (e = pair for Q, 4+pair for K)
                    for e in range(2 * NPAIR):
                        ps = ps1.tile([128, 512], F32, tag="qkps", bufs=3)
                        for d in range(ND):
                            nc.tensor.matmul(
                                ps[:], w_sb[d][:, e * 128:(e + 1) * 128],
                                xa[:, d, :],
                                start=(d == 0), stop=(d == ND - 1),
                            )
                        dst = qt[e] if e < NPAIR else kt[e - NPAIR]
                        nc.any.tensor_copy(dst[:, tsl], ps[:])
                        # RoPE in place: quadrant swap via SBUF->SBUF DMA
                        sw = ropep.tile([128, 512], F32, tag="sw", bufs=2)
                        for qd in range(4):
                            sq = qd ^ 1
                            nc.gpsimd.dma_start(
                                sw[qd * 32:(qd + 1) * 32, :],
                                dst.bitcast(F32)[sq * 32:(sq + 1) * 32, tsl],
                            )
                        t1 = ropep.tile([128, 512], F32, tag="t1", bufs=2)
                        nc.vector.tensor_mul(t1[:], dst[:, tsl], cos_sb[:])
                        nc.vector.tensor_mul(sw[:], sw[:], sin_sb[:])
                        nc.vector.tensor_add(dst[:, tsl], t1[:], sw[:])

                    # V (natural layout), 4 t-chunks of 128 per ts
                    for tq0 in range(4):
                        tq = ts * 4 + tq0
                        psv = ps1.tile([128, 512], F32, tag="qkps", bufs=3)
                        for d in range(ND):
                            nc.tensor.matmul(
                                psv[:],
                                xa[:, d, tq0 * 128:(tq0 + 1) * 128],
                                w_sb[d][:, 2 * HLOC * DH:3 * HLOC * DH],
                                start=(d == 0), stop=(d == ND - 1),
                            )
                        v = vt[tq]
                        nc.any.tensor_copy(
                            v[:, :, 0:DH],
                            psv.rearrange("p (h d) -> p h d", h=HLOC),
                        )
                        nc.any.tensor_copy(v[:, :, DH:DH + 1], ones8[:, :, None])

            with tc.tile_pool(name="ctxbp", bufs=1) as ctxbp:
                # ctx homes for odd heads; even heads reuse dead qt columns
                ctxb = [ctxbp.tile([65, S], F32R, name=f"ctxb{p}")
                        for p in range(NPAIR)]

                def ctx_home(h):
                    p = h // 2
                    return qt[p] if h % 2 == 0 else ctxb[p]

                # ---------------- Phase 2: attention ----------------
                with (
                    tc.tile_pool(name="ptpool", bufs=1) as ptpool,
                    tc.tile_pool(name="nrmpool", bufs=1) as nrmpool,
                    tc.tile_pool(name="stp", bufs=1, space="PSUM") as stp,
                    tc.tile_pool(name="cdp", bufs=1, space="PSUM") as cdp,
                ):
                    onehot_sb = nrmpool.tile([8, 8 * 64], F32R)
                    nc.sync.dma_start(onehot_sb[:], onehot_d[:])
                    for i in range(NT):
                        for p in range(NPAIR):
                            ctxA = cdp.tile([65, 512], F32, tag="ctx", bufs=4)
                            ctxB = cdp.tile([65, 512], F32, tag="ctx", bufs=4)
                            nj = 4 * i + 4
                            for j in range(nj):
                                lo = max(0, 128 * j - 512 * i)
                                qsl = slice(512 * i + lo, 512 * (i + 1))
                                ksl = slice(j * 128, (j + 1) * 128)
                                st = stp.tile([128, 2, 512], F32, tag="st", bufs=2)
                                nc.tensor.matmul(
                                    st[:, 0, lo:512], kt[p][0:64, ksl],
                                    qt[p][0:64, qsl], tile_position=(0, 0),
                                )
                                nc.tensor.matmul(
                                    st[:, 1, lo:512], kt[p][64:128, ksl],
                                    qt[p][64:128, qsl], tile_position=(64, 0),
                                )
                                pt = ptpool.tile([128, 2, 512], F32R, tag="pt", bufs=6)
                                nc.scalar.activation(
                                    pt[:, :, lo:512], st[:, :, lo:512], EXP,
                                    scale=SCALE,
                                )
                                if lo == 128 * j - 512 * i:  # starts on diagonal
                                    nc.vector.tensor_mul(
                                        pt[:, :, lo:lo + 128],
                                        pt[:, :, lo:lo + 128],
                                        mask_sb[:, None, :].to_broadcast([128, 2, 128]),
                                    )
                                nc.tensor.matmul(
                                    ctxA[:, lo:512], vt[j][:, 2 * p, :],
                                    pt[:, 0, lo:512],
                                    start=(j == 0), stop=(j == nj - 1),
                                )
                                nc.tensor.matmul(
                                    ctxB[:, lo:512], vt[j][:, 2 * p + 1, :],
                                    pt[:, 1, lo:512],
                                    start=(j == 0), stop=(j == nj - 1),
                                )
                            isl = slice(512 * i, 512 * (i + 1))
                            # stash unnormalized ctx + denominator row
                            nc.vector.tensor_copy(qt[p][0:65, isl], ctxA[:])
                            nc.vector.tensor_copy(ctxb[p][:, isl], ctxB[:])

                        # per-q-tile softmax normalization for row i, emitted
                        # inline so it overlaps the next attention row
                        tsl = isl
                        den_g = nrmpool.tile([8, 512], F32R, tag="deng", bufs=2,
                                             name=f"deng{i}")
                        for h in range(HLOC):
                            nc.sync.dma_start(
                                den_g[h:h + 1, :], ctx_home(h)[64:65, tsl])
                        rec = nrmpool.tile([8, 512], F32, tag="recg", bufs=2,
                                           name=f"rec{i}")
                        nc.vector.reciprocal_approx_fast(
                            rec[:], den_g.bitcast(F32)[:])
                        rec_r = nrmpool.tile([8, 512], F32R, tag="recr", bufs=2,
                                             name=f"recr{i}")
                        nc.vector.tensor_copy(rec_r[:], rec[:])
                        for h in range(HLOC):
                            home = ctx_home(h)
                            bc = cdp.tile([64, 512], F32, tag="ctx", bufs=4,
                                          name=f"bc{i}_{h}")
                            nc.tensor.matmul(
                                bc[:], onehot_sb[:, h * 64:(h + 1) * 64], rec_r[:])
                            bc_sb = nrmpool.tile([64, 512], F32, tag="bcsb", bufs=4)
                            nc.any.tensor_copy(bc_sb[:], bc[:])
                            nc.vector.tensor_mul(
                                home[0:64, tsl], home[0:64, tsl], bc_sb[:])
                        # repack odd-head ctx into dead Q_B rows (K=128 out-proj)
                        for p in range(NPAIR):
                            nc.sync.dma_start(
                                qt[p][64:128, tsl], ctxb[p][0:64, tsl])

                # ---------- Phase 3: out projection (outT layout) ----------
                with (
                    tc.tile_pool(name="wopool", bufs=1) as wopool,
                    tc.tile_pool(name="otpool", bufs=1) as otpool,
                    tc.tile_pool(name="pso", bufs=1, space="PSUM") as psop,
                ):
                    wo_sb = []
                    for p in range(NPAIR):
                        wo = wopool.tile([128, D], F32R, name=f"wo{p}")
                        nc.sync.dma_start(wo[:], woT_d[p])
                        wo_sb.append(wo)

                    for ts in range(NT):
                        tsl = slice(ts * 512, (ts + 1) * 512)
                        for ec in range(D // 128):
                            ecs = slice(ec * 128, (ec + 1) * 128)
                            pso = psop.tile([128, 512], F32, tag="o", bufs=6,
                                            name=f"pso_{ts}_{ec}")
                            for p in range(NPAIR):
                                nc.tensor.matmul(
                                    pso[:], wo_sb[p][:, ecs], qt[p][:, tsl],
                                    start=(p == 0), stop=(p == NPAIR - 1),
                                )
                            ot = otpool.tile([128, 512], F32, tag="ot", bufs=4)
                            nc.vector.tensor_copy(ot[:], pso[:])
                            nc.sync.dma_start(out_d[ecs, tsl], ot[:])

    nc.compile()
    return nc


def _get_program():
    global _PROGRAM
    if _PROGRAM is None:
        _PROGRAM = _build_program()
    return _PROGRAM


def _prep_in_maps(in_features, token_positions, W_qkv, W_out):
    in_features = np.asarray(in_features, dtype=np.float32)
    token_positions = np.asarray(token_positions)
    W_qkv = np.asarray(W_qkv, dtype=np.float32)
    W_out = np.asarray(W_out, dtype=np.float32)

    # RoPE pair permutation: [x0 of freq 0..31 | x1 of freq 0..31]
    perm = np.concatenate([np.arange(0, DH, 2), np.arange(1, DH, 2)])

    wqkvT, woT = [], []
    for tp in range(TP):
        rows = []
        for sect in range(2):  # Q, K (permuted)
            for h in range(HLOC):
                g = tp * HLOC + h
                rows.append(W_qkv[sect * D + g * DH + perm])
        for h in range(HLOC):  # V natural
            g = tp * HLOC + h
            rows.append(W_qkv[2 * D + g * DH:2 * D + (g + 1) * DH])
        Wl = np.concatenate(rows, axis=0)  # [1536, 1024]
        wqkvT.append(np.ascontiguousarray(Wl.T))
        woT.append(np.ascontiguousarray(np.stack(
            [np.concatenate([
                W_out[:, (tp * HLOC + 2 * p) * DH:(tp * HLOC + 2 * p + 1) * DH].T,
                W_out[:, (tp * HLOC + 2 * p + 1) * DH:(tp * HLOC + 2 * p + 2) * DH].T,
            ], axis=0) for p in range(NPAIR)])))

    half = DH // 2
    inv_freq = (THETA ** (-2.0 * np.arange(half, dtype=np.float32) / DH)).astype(np.float32)
    ang = token_positions.astype(np.float32)[:, None] * inv_freq[None, :]  # [S, 32]
    cos_t = np.cos(ang).T.astype(np.float32)  # [32, S]
    sin_t = np.sin(ang).T.astype(np.float32)
    cos128 = np.ascontiguousarray(np.tile(cos_t, (4, 1)))
    sin128 = np.ascontiguousarray(np.tile(np.concatenate([-sin_t, sin_t], axis=0), (2, 1)))
    # mask[kv, c] = 1 iff kv <= c (scores stored transposed: [kv, q])
    mask128 = np.triu(np.ones((128, 128), dtype=np.float32))
    onehot = np.zeros((8, 8 * 64), dtype=np.float32)
    for k in range(8):
        onehot[k, k * 64:(k + 1) * 64] = 1.0

    in_maps = []
    for c in range(NCORES):
        b, tp = c // 2, c % 2
        in_maps.append({
            "xT": np.ascontiguousarray(in_features[b].T),
            "wqkvT": wqkvT[tp],
            "woT": woT[tp],
            "cosT": cos128,
            "sinT": sin128,
            "mask": mask128,
            "onehot": onehot,
        })
    return in_maps


def run(in_features, token_positions, W_qkv, W_out, **spmd_kwargs):
    """Run the kernel; returns (output [B,S,D] f32, BassKernelResults)."""
    in_maps = _prep_in_maps(in_features, token_positions, W_qkv, W_out)
    nc = _get_program()
    res = run_bass_kernel_spmd(nc, in_maps, core_ids=list(range(NCORES)), **spmd_kwargs)
    outs = [res.results[c]["out"] for c in range(NCORES)]
    full = np.stack([(outs[2 * b] + outs[2 * b + 1]).T for b in range(B)])
    return full.astype(np.float32), res


def kernel(in_features, token_positions, W_qkv, W_out):
    out, _ = run(in_features, token_positions, W_qkv, W_out)
    return out
